# revision 14
# baseline (speedup 1.0000x reference)
"""GCN (3x GraphSAGE + gated skip) on 8 Trainium2 NeuronCores.

Strategy (self-contained, hardcoded for N=50000, E=1.6M, NFEAT=256, NHID=128):
- Nodes sharded 6250/core; edges partitioned by destination core.
- Per core, dests are permuted by degree (descending) so the per-block
  gather slot grids are tight; edge sources split into halves A (global
  permuted row < 25000) / B (>= 25000) so gather indices fit int16.
- Aggregation: dma_gather (4 SWDGE queues) of bf16 projected tables
  (xp = x @ W_bot), accumulated per 128-dest block on TensorE with an
  identity-stationary matmul into PSUM (f32).
- B-half aggregated in its own degree order, written to DRAM f32, then
  permuted into main order with one more (cheap) dma_gather.
- Per-layer full tables produced by AllGather of per-shard projections.
- Node pipeline: bf16 matmul inputs, f32 PSUM/elementwise.
"""

import os
import numpy as np
import ml_dtypes

import concourse.bacc as bacc
import concourse.mybir as mybir
import concourse.tile as tile
import concourse.bass as bass
from concourse.bass_utils import run_bass_kernel_spmd
from concourse.masks import make_identity

# ----- Tile SWDGE-queue patch: Tile round-robins the 8 DMASW completion-sem
# lanes ignoring the SWDGE queue; HW needs each sem fed by one queue only.
# Partition lanes by queue: queue q owns lanes {2q, 2q+1}.
import concourse.tile_sem_assignment as _tsa

_orig_assign_tick = _tsa.TileClockTick._assign_tick


def _assign_tick_queue_aware(self, inst):
    if (
        isinstance(inst, _tsa.DMAInst)
        and inst.engine == _tsa.mybir.EngineType.Pool
        and not isinstance(inst, _tsa.bass_isa.UserSyncedRemoteDMADescs)
    ):
        q = getattr(inst, "queue_num", 0) or 0
        if not hasattr(self, "_q_sub"):
            self._q_sub = {}
        sub = self._q_sub.get(q, 0)
        self.next_sw_dma_idx = (2 * q + sub) % 8
        self._q_sub[q] = sub ^ 1
    return _orig_assign_tick(self, inst)


if os.environ.get("BASS_GCN_NOPATCH", "0") != "1":
    if getattr(_tsa.TileClockTick._assign_tick, "__name__", "") != "_assign_tick_queue_aware":
        _tsa.TileClockTick._assign_tick = _assign_tick_queue_aware

# ---------------- problem constants ----------------
N = 50000
E = 1_600_000
NFEAT = 256
NHID = 128
NCORES = 8
ND = N // NCORES          # 6250 dests per core
NB = 49                   # 128-dest blocks per core (last partial)
NDP = NB * 128            # 6272 padded dests
HALF = 25000              # global permuted-row split for int16 indices
TROWS = N + 2             # table rows: [zero, 50000 nodes, zero]
WSLOTS = 8                # max slots (x128 idxs) per dma_gather window
NQ = 4                    # SWDGE queues
DT_B = mybir.dt.bfloat16
DT_F = mybir.dt.float32
DEBUG = os.environ.get("BASS_GCN_DEBUG", "0") == "1"


def _wrap_window(flat):
    """[n] -> [128, n//16] int16 in the dma_gather idx layout (wrapped in 16
    partitions, replicated across the 8 Q7 core groups)."""
    n = flat.size
    blk = flat.reshape(n // 16, 16).T.astype(np.int16)  # [16, n//16]
    return np.tile(blk, (8, 1))


def _make_windows(K):
    """Split the slot stream [(block, s) for b for s in range(K[b])] into
    windows of <= WSLOTS slots. Returns list of windows; each window is a
    list of (block, s, is_first, is_last)."""
    slots = []
    for b in range(NB):
        for s in range(K[b]):
            slots.append((b, s, s == 0, s == K[b] - 1))
    return [slots[i : i + WSLOTS] for i in range(0, len(slots), WSLOTS)]


def preprocess(row, col):
    """Host-side graph preprocessing. Returns (plan, per_core_inputs)."""
    row = np.asarray(row).astype(np.int64)
    col = np.asarray(col).astype(np.int64)
    deg = np.bincount(row, minlength=N)
    inv_deg = (1.0 / np.maximum(deg, 1.0)).astype(np.float32)

    owner = row // ND
    ldest = row - owner * ND
    isA_src = col < (4 * ND)  # source owned by cores 0-3
    degA = np.bincount(row[isA_src], minlength=N)
    degB = deg - degA

    # per-core degree-descending permutations
    piA = np.empty((NCORES, ND), np.int64)   # rank -> local node (main order)
    rkA = np.empty((NCORES, ND), np.int64)   # local node -> rank
    piB = np.empty((NCORES, ND), np.int64)
    rkB = np.empty((NCORES, ND), np.int64)
    for c in range(NCORES):
        dA = degA[c * ND : (c + 1) * ND]
        dB = degB[c * ND : (c + 1) * ND]
        piA[c] = np.argsort(-dA, kind="stable")
        piB[c] = np.argsort(-dB, kind="stable")
        rkA[c, piA[c]] = np.arange(ND)
        rkB[c, piB[c]] = np.arange(ND)

    # source table index (in the permuted-global numbering)
    src_owner = col // ND
    src_pos = src_owner * ND + rkA[src_owner, col - src_owner * ND]
    # view index within half (+1 for viewA's leading zero row)
    src_view = np.where(src_pos < HALF, src_pos + 1, src_pos - HALF)

    # uniform per-block slot-count profiles (max over cores; desc sort =>
    # per-core block max is the first rank of the block)
    KA = np.zeros(NB, np.int64)
    KB = np.zeros(NB, np.int64)
    for c in range(NCORES):
        dA_sorted = degA[c * ND : (c + 1) * ND][piA[c]]
        dB_sorted = degB[c * ND : (c + 1) * ND][piB[c]]
        for b in range(NB):
            KA[b] = max(KA[b], dA_sorted[b * 128] if b * 128 < ND else 0)
            KB[b] = max(KB[b], dB_sorted[b * 128] if b * 128 < ND else 0)
    KA = [int(k) for k in KA]
    KB = [int(k) for k in KB]

    winA = _make_windows(KA)
    winB = _make_windows(KB)
    # merge windows over NDP main-rank positions
    mlens = []
    rem = NDP
    while rem > 0:
        mlens.append(min(rem, 1024))
        rem -= mlens[-1]

    per_core = []
    for c in range(NCORES):
        m = owner == c
        er = rkA[c, ldest[m]]          # dest main-rank of this core's edges
        ev = src_view[m]
        eA = isA_src[m]
        erB = rkB[c, ldest[m]]

        def dense(ranks, vals, K_prof, pad):
            D = np.full((NDP, max(K_prof) if K_prof else 1), pad, np.int64)
            o = np.argsort(ranks, kind="stable")
            rs = ranks[o]
            vs = vals[o]
            # slot index within each dest
            first = np.zeros(len(rs), bool)
            if len(rs):
                first[0] = True
                first[1:] = rs[1:] != rs[:-1]
            starts = np.flatnonzero(first)
            counts = np.diff(np.append(starts, len(rs)))
            slot = np.arange(len(rs)) - np.repeat(starts, counts)
            D[rs, slot] = vs
            return D

        DA = dense(er[eA], ev[eA], KA, 0)
        DB = dense(erB[~eA], ev[~eA], KB, HALF)

        cols = []
        for win in winB:
            idxs = np.concatenate([DB[b * 128 : b * 128 + 128, s] for (b, s, _, _) in win])
            cols.append(_wrap_window(idxs))
        for win in winA:
            idxs = np.concatenate([DA[b * 128 : b * 128 + 128, s] for (b, s, _, _) in win])
            cols.append(_wrap_window(idxs))
        # merge stream: main rank j -> aggB row rkB[piA[j]]
        mj = np.zeros(NDP, np.int64)
        mj[:ND] = rkB[c, piA[c]]
        off = 0
        for L in mlens:
            cols.append(_wrap_window(mj[off : off + L]))
            off += L
        grid = np.concatenate(cols, axis=1)

        invd = np.ones(NDP, np.float32)
        invd[:ND] = inv_deg[c * ND : (c + 1) * ND][piA[c]]
        per_core.append(
            {
                "idxs": grid,
                "invdeg": np.ascontiguousarray(invd.reshape(NB, 128).T),
            }
        )

    plan = {
        "winA": winA,
        "winB": winB,
        "mlens": mlens,
        "KA": KA,
        "KB": KB,
        "grid_cols": per_core[0]["idxs"].shape[1],
        "piA": piA,
    }
    return plan, per_core


def build(plan):
    nc = bacc.Bacc("TRN2", debug=False, num_swdge_queues=NQ)
    x_in = nc.dram_tensor("x", [NDP, NFEAT], DT_F, kind="ExternalInput")
    idxs_in = nc.dram_tensor("idxs", [128, plan["grid_cols"]], mybir.dt.int16, kind="ExternalInput")
    invdeg_in = nc.dram_tensor("invdeg", [128, NB], DT_F, kind="ExternalInput")
    w_in = {}
    for name, shape in [
        ("W_sc", [NFEAT, NHID]), ("W_ci", [NHID, NHID]), ("W_co", [NHID, NHID]),
        ("W0", [2 * NFEAT, NHID]), ("W1", [2 * NHID, NHID]), ("W2", [2 * NHID, NHID]),
        ("Wf", [NHID, NHID]),
        ("b_ci", [1, NHID]), ("b_co", [1, NHID]), ("b0", [1, NHID]),
        ("b1", [1, NHID]), ("b2", [1, NHID]), ("bf", [1, NHID]),
    ]:
        w_in[name] = nc.dram_tensor(name, shape, DT_F, kind="ExternalInput")
    out_t = nc.dram_tensor("out", [NDP, NHID], DT_F, kind="ExternalOutput")
    dbg = {}
    if DEBUG:
        for nm in ("dbg_o1", "dbg_o3"):
            dbg[nm] = nc.dram_tensor(nm, [NDP, NHID], DT_F, kind="ExternalOutput")

    tables = [
        nc.dram_tensor(f"table{l}", [TROWS, NHID], DT_B, kind="Internal", addr_space="Shared")
        for l in range(3)
    ]
    ag_ins = [nc.dram_tensor(f"ag_in{l}", [ND, NHID], DT_B, kind="Internal") for l in range(3)]
    aggBs = [nc.dram_tensor(f"aggB{l}", [NDP, NHID], DT_F, kind="Internal") for l in range(3)]

    winA, winB, mlens = plan["winA"], plan["winB"], plan["mlens"]
    KA, KB = plan["KA"], plan["KB"]
    n_gw = len(winA) + len(winB)

    with tile.TileContext(nc) as tc:
        with (
            tc.tile_pool(name="const", bufs=1) as cpool,
            tc.tile_pool(name="persist", bufs=1) as ppool,
            tc.tile_pool(name="idx", bufs=1) as ipool,
            tc.tile_pool(name="g", bufs=8) as gpool,
            tc.tile_pool(name="gm", bufs=3) as mpool,
            tc.tile_pool(name="blk", bufs=4) as bpool,
            tc.tile_pool(name="xload", bufs=3) as xpool,
            tc.tile_pool(name="ps_agg", bufs=4, space="PSUM") as ps_agg,
            tc.tile_pool(name="ps_mm", bufs=2, space="PSUM") as ps_mm,
            tc.tile_pool(name="ps_tr", bufs=2, space="PSUM") as ps_tr,
        ):
            # ---------------- constants / weights ----------------
            ident_b = cpool.tile([128, 128], DT_B)
            make_identity(nc, ident_b[:])
            zrow = cpool.tile([1, NHID], DT_B)
            nc.vector.memset(zrow[:], 0.0)
            for t in tables:
                nc.sync.dma_start(t[0:1, :], zrow[:])
                nc.sync.dma_start(t[N + 1 : N + 2, :], zrow[:])

            # weights -> SBUF bf16 (cast during SWDGE DMA), stored as
            # [128, n_ktiles, NHID]; k-tile i = wb[name][:, i, :]
            wb = {}
            for name in ("W_sc", "W_ci", "W_co", "W0", "W1", "W2", "Wf"):
                r = w_in[name].shape[0]
                kt = r // 128
                tf = cpool.tile([128, kt, NHID], DT_F, tag="wload", bufs=2,
                                name=f"wl_{name}")
                nc.sync.dma_start(
                    tf[:], w_in[name][:].rearrange("(k p) c -> p k c", p=128)
                )
                t = cpool.tile([128, kt, NHID], DT_B, name=f"wb_{name}")
                nc.vector.tensor_copy(t[:], tf[:])
                wb[name] = t
            # biases f32 rows
            brow = {}
            for name in ("b_ci", "b_co", "b0", "b1", "b2", "bf"):
                t = cpool.tile([1, NHID], DT_F, name=f"br_{name}")
                nc.sync.dma_start(t[:], w_in[name][:])
                brow[name] = t
            bg_row = cpool.tile([1, NHID], DT_F)
            nc.vector.tensor_add(bg_row[:], brow["b_ci"][:], brow["b_co"][:])
            ones_col = cpool.tile([1, 128], DT_B)
            nc.vector.memset(ones_col[:], 1.0)

            _repc = [0]

            def replicate_row(row_f32):
                _repc[0] += 1
                ps = ps_mm.tile([128, NHID], DT_F, space="PSUM", tag="mm",
                                name=f"repps_{_repc[0]}")
                rb = cpool.tile([1, NHID], DT_B, name=f"rb_{_repc[0]}")
                nc.vector.tensor_copy(rb[:], row_f32[:])
                nc.tensor.matmul(out=ps[:], lhsT=ones_col[:], rhs=rb[:], start=True, stop=True)
                t = cpool.tile([128, NHID], DT_F, name=f"rep_{_repc[0]}")
                nc.vector.tensor_copy(t[:], ps[:])
                return t

            brep = {k: replicate_row(brow[k]) for k in ("b0", "b1", "b2", "bf")}
            bg_rep = replicate_row(bg_row)

            invdeg_sb = ppool.tile([128, NB], DT_F)
            nc.sync.dma_start(invdeg_sb[:], invdeg_in[:])

            idx_sb = ipool.tile([128, plan["grid_cols"]], mybir.dt.int16)
            nc.sync.dma_start(idx_sb[:], idxs_in[:])

            # W_sci = W_sc @ W_ci  (via PE transpose of W_sc)
            wsci = cpool.tile([128, 2, NHID], DT_B)
            for i in range(2):
                pst = ps_tr.tile([128, 128], DT_B, space="PSUM", tag="tr")
                nc.tensor.transpose(out=pst[:], in_=wb["W_sc"][:, i, :], identity=ident_b[:])
                wsct = cpool.tile([128, 128], DT_B)
                nc.vector.tensor_copy(wsct[:], pst[:])
                ps = ps_mm.tile([128, NHID], DT_F, space="PSUM", tag="mm")
                nc.tensor.matmul(out=ps[:], lhsT=wsct[:], rhs=wb["W_ci"][:, 0, :], start=True, stop=True)
                nc.vector.tensor_copy(wsci[:, i, :], ps[:])

            # ---------------- head: per-block x projections ----------------
            xs_sb = ppool.tile([128, NB * 128], DT_F)      # x @ W_sc
            cwb_sb = ppool.tile([128, NB * 128], DT_B)     # x @ W_sci + (b_ci+b_co)
            otop_sb = ppool.tile([128, NB * 128], DT_F)    # x @ W0_top (later o@W_top)
            aggA_sb = ppool.tile([128, NB * 128], DT_F)

            for b in range(NB):
                xt = xpool.tile([128, NFEAT], DT_F)
                nc.sync.dma_start(xt[:], x_in[b * 128 : (b + 1) * 128, :])
                xtb = xpool.tile([128, NFEAT], DT_B)
                nc.vector.tensor_copy(xtb[:], xt[:])
                xT = []
                for i in range(2):
                    pst = ps_tr.tile([128, 128], DT_B, space="PSUM", tag="tr")
                    nc.tensor.transpose(out=pst[:], in_=xtb[:, i * 128 : (i + 1) * 128], identity=ident_b[:])
                    tt = xpool.tile([128, 128], DT_B, tag="xT")
                    nc.vector.tensor_copy(tt[:], pst[:])
                    xT.append(tt)

                def proj2(rhs0, rhs1):
                    ps = ps_mm.tile([128, NHID], DT_F, space="PSUM", tag="mm")
                    nc.tensor.matmul(out=ps[:], lhsT=xT[0][:], rhs=rhs0, start=True, stop=False)
                    nc.tensor.matmul(out=ps[:], lhsT=xT[1][:], rhs=rhs1, start=False, stop=True)
                    return ps

                bs = slice(b * 128, (b + 1) * 128)
                ps = proj2(wb["W_sc"][:, 0, :], wb["W_sc"][:, 1, :])
                nc.vector.tensor_copy(xs_sb[:, bs], ps[:])
                ps = proj2(wsci[:, 0, :], wsci[:, 1, :])
                nc.vector.tensor_add(cwb_sb[:, bs], ps[:], bg_rep[:])
                ps = proj2(wb["W0"][:, 0, :], wb["W0"][:, 1, :])
                nc.vector.tensor_copy(otop_sb[:, bs], ps[:])
                ps = proj2(wb["W0"][:, 2, :], wb["W0"][:, 3, :])
                xpb = xpool.tile([128, NHID], DT_B, tag="xpb")
                nc.vector.tensor_copy(xpb[:], ps[:])
                lo = b * 128
                hi = min((b + 1) * 128, ND)
                nc.sync.dma_start(ag_ins[0][lo:hi, :], xpb[0 : hi - lo, :])

            nc.gpsimd.collective_compute(
                "AllGather", mybir.AluOpType.bypass,
                replica_groups=[list(range(NCORES))],
                ins=[ag_ins[0][:]], outs=[tables[0][1 : N + 1, :]],
            )

            # ---------------- layers ----------------
            gq = [0]  # rotating queue counter

            def gather(dst_ap, src_ap, col_off, n_idxs):
                q = gq[0] % NQ
                gq[0] += 1
                nc.gpsimd.dma_gather(
                    dst_ap, src_ap,
                    idx_sb[:, col_off // 16 : (col_off + n_idxs) // 16],
                    n_idxs, n_idxs, NHID, queue_num=q,
                )

            for l in range(3):
                table = tables[l]
                viewA = table[0 : HALF + 1, :]
                viewB = table[HALF + 1 : TROWS, :]
                W_next = {0: wb["W1"], 1: wb["W2"]}.get(l)  # weights of layer l+1

                col = [0]  # grid column cursor (in idx units)

                def run_half(windows, K, view, sb_dst, to_dram):
                    """Gather+accumulate one half. sb_dst: aggA_sb or None;
                    to_dram: aggB dram tensor or None."""
                    ps_cur = {}
                    for win in windows:
                        nsl = len(win)
                        g = gpool.tile([128, WSLOTS, NHID], DT_B)
                        gather(g[:, 0:nsl, :], view, col[0], nsl * 128)
                        col[0] += nsl * 128
                        for t, (b, s, first, last) in enumerate(win):
                            if first:
                                ps_cur[b] = ps_agg.tile([128, NHID], DT_F, space="PSUM", tag="psagg", name=f"psagg_{id(ps_cur)}_{b}")
                            nc.tensor.matmul(
                                out=ps_cur[b][:], lhsT=ident_b[:], rhs=g[:, t, :],
                                start=first, stop=last,
                            )
                            if last:
                                if sb_dst is not None:
                                    nc.vector.tensor_copy(sb_dst[:, b * 128 : (b + 1) * 128], ps_cur[b][:])
                                else:
                                    ob = bpool.tile([128, NHID], DT_F, tag="aggBblk")
                                    nc.vector.tensor_copy(ob[:], ps_cur[b][:])
                                    nc.sync.dma_start(to_dram[b * 128 : (b + 1) * 128, :], ob[:])
                                del ps_cur[b]
                    for b in range(NB):
                        if K[b] == 0:
                            if sb_dst is not None:
                                nc.vector.memset(sb_dst[:, b * 128 : (b + 1) * 128], 0.0)
                            else:
                                ob = bpool.tile([128, NHID], DT_F, tag="aggBblk")
                                nc.vector.memset(ob[:], 0.0)
                                nc.sync.dma_start(to_dram[b * 128 : (b + 1) * 128, :], ob[:])

                run_half(winB, KB, viewB, None, aggBs[l])
                run_half(winA, KA, viewA, aggA_sb, None)

                # permuted readback of aggB into main order
                mtiles = []
                moff = 0
                for L in mlens:
                    gm = mpool.tile([128, WSLOTS, NHID], DT_F)
                    gather(gm[:, 0 : L // 128, :], aggBs[l][:], col[0], L)
                    col[0] += L
                    for t in range(L // 128):
                        mtiles.append((gm, t))
                    moff += L

                # ------------- per-block assembly + GSC + tail -------------
                for b in range(NB):
                    bs = slice(b * 128, (b + 1) * 128)
                    gm, t = mtiles[b]
                    o_sb = bpool.tile([128, NHID], DT_F, tag="o")
                    # t1 = aggA + aggBm ; t2 = t1 * invdeg ; o = t2 + otop + b_l
                    nc.vector.tensor_add(o_sb[:], aggA_sb[:, bs], gm[:, t, :])
                    nc.vector.tensor_scalar(
                        out=o_sb[:], in0=o_sb[:],
                        scalar1=invdeg_sb[:, b : b + 1], scalar2=None,
                        op0=mybir.AluOpType.mult,
                    )
                    nc.vector.tensor_add(o_sb[:], o_sb[:], otop_sb[:, bs])
                    nc.vector.tensor_add(o_sb[:], o_sb[:], brep[{0: "b0", 1: "b1", 2: "b2"}[l]][:])
                    if DEBUG and l == 0:
                        nc.sync.dma_start(dbg["dbg_o1"][bs, :], o_sb[:])

                    # GSC: z = sigmoid(o @ W_co + cwb); g = z*o + (1-z)*xs
                    ob16 = bpool.tile([128, NHID], DT_B, tag="ob16")
                    nc.vector.tensor_copy(ob16[:], o_sb[:])
                    pst = ps_tr.tile([128, 128], DT_B, space="PSUM", tag="tr")
                    nc.tensor.transpose(out=pst[:], in_=ob16[:], identity=ident_b[:])
                    oT = bpool.tile([128, NHID], DT_B, tag="oT")
                    nc.vector.tensor_copy(oT[:], pst[:])
                    psz = ps_mm.tile([128, NHID], DT_F, space="PSUM", tag="mm")
                    nc.tensor.matmul(out=psz[:], lhsT=oT[:], rhs=wb["W_co"][:, 0, :], start=True, stop=True)
                    v = bpool.tile([128, NHID], DT_F, tag="v")
                    nc.vector.tensor_add(v[:], psz[:], cwb_sb[:, bs])
                    z = bpool.tile([128, NHID], DT_F, tag="z")
                    nc.scalar.activation(z[:], v[:], mybir.ActivationFunctionType.Sigmoid)
                    u = bpool.tile([128, NHID], DT_F, tag="u")
                    nc.vector.tensor_tensor(out=u[:], in0=o_sb[:], in1=xs_sb[:, bs], op=mybir.AluOpType.subtract)
                    nc.vector.tensor_tensor(out=u[:], in0=z[:], in1=u[:], op=mybir.AluOpType.mult)
                    gsc = bpool.tile([128, NHID], DT_F, tag="gsc")
                    nc.vector.tensor_add(gsc[:], u[:], xs_sb[:, bs])
                    if l < 2:
                        nc.scalar.activation(gsc[:], gsc[:], mybir.ActivationFunctionType.Relu)
                    if DEBUG and l == 0:
                        nc.sync.dma_start(dbg["dbg_o3"][bs, :], gsc[:])

                    # tail: transpose gsc; project for next layer or final
                    g16 = bpool.tile([128, NHID], DT_B, tag="g16")
                    nc.vector.tensor_copy(g16[:], gsc[:])
                    pst2 = ps_tr.tile([128, 128], DT_B, space="PSUM", tag="tr")
                    nc.tensor.transpose(out=pst2[:], in_=g16[:], identity=ident_b[:])
                    gT = bpool.tile([128, NHID], DT_B, tag="gT")
                    nc.vector.tensor_copy(gT[:], pst2[:])
                    if l < 2:
                        pj = ps_mm.tile([128, NHID], DT_F, space="PSUM", tag="mm")
                        nc.tensor.matmul(out=pj[:], lhsT=gT[:], rhs=W_next[:, 1, :], start=True, stop=True)
                        tb16 = bpool.tile([128, NHID], DT_B, tag="tb16")
                        nc.vector.tensor_copy(tb16[:], pj[:])
                        lo = b * 128
                        hi = min((b + 1) * 128, ND)
                        nc.sync.dma_start(ag_ins[l + 1][lo:hi, :], tb16[0 : hi - lo, :])
                        pj2 = ps_mm.tile([128, NHID], DT_F, space="PSUM", tag="mm")
                        nc.tensor.matmul(out=pj2[:], lhsT=gT[:], rhs=W_next[:, 0, :], start=True, stop=True)
                        nc.vector.tensor_copy(otop_sb[:, bs], pj2[:])
                    else:
                        pf = ps_mm.tile([128, NHID], DT_F, space="PSUM", tag="mm")
                        nc.tensor.matmul(out=pf[:], lhsT=gT[:], rhs=wb["Wf"][:, 0, :], start=True, stop=True)
                        res = bpool.tile([128, NHID], DT_F, tag="res")
                        nc.vector.tensor_add(res[:], pf[:], brep["bf"][:])
                        nc.sync.dma_start(out_t[b * 128 : (b + 1) * 128, :], res[:])

                if l < 2:
                    nc.gpsimd.collective_compute(
                        "AllGather", mybir.AluOpType.bypass,
                        replica_groups=[list(range(NCORES))],
                        ins=[ag_ins[l + 1][:]], outs=[tables[l + 1][1 : N + 1, :]],
                    )

    nc.compile()
    return nc


def _pack_weights(inp):
    w = {}
    for k in ("W_sc", "W_ci", "W_co", "W0", "W1", "W2", "Wf"):
        w[k] = np.ascontiguousarray(inp[k], np.float32)
    for k in ("b_ci", "b_co", "b0", "b1", "b2", "bf"):
        w[k] = np.ascontiguousarray(np.asarray(inp[k]).reshape(1, NHID), np.float32)
    return w


_CACHE = {}


def kernel(x, row, col, W_sc, W_ci, b_ci, W_co, b_co,
           W0, b0, W1, b1, W2, b2, Wf, bf):
    key = (int(np.asarray(row)[:64].sum()), int(np.asarray(col)[:64].sum()))
    if key not in _CACHE:
        plan, per_core = preprocess(row, col)
        nc = build(plan)
        _CACHE[key] = (plan, per_core, nc)
    plan, per_core, nc = _CACHE[key]
    piA = plan["piA"]

    x = np.asarray(x, np.float32)
    weights = _pack_weights(dict(
        W_sc=W_sc, W_ci=W_ci, b_ci=b_ci, W_co=W_co, b_co=b_co, W0=W0, b0=b0,
        W1=W1, b1=b1, W2=W2, b2=b2, Wf=Wf, bf=bf))

    in_maps = []
    for c in range(NCORES):
        xs = np.zeros((NDP, NFEAT), np.float32)
        xs[:ND] = x[c * ND : (c + 1) * ND][piA[c]]
        m = dict(weights)
        m["x"] = xs
        m["idxs"] = per_core[c]["idxs"]
        m["invdeg"] = per_core[c]["invdeg"]
        in_maps.append(m)

    res = run_bass_kernel_spmd(nc, in_maps, core_ids=list(range(NCORES)), trace=False)
    out = np.empty((N, NHID), np.float32)
    for c in range(NCORES):
        shard = res.results[c]["out"][:ND]
        full = np.empty((ND, NHID), np.float32)
        full[piA[c]] = shard
        out[c * ND : (c + 1) * ND] = full
    return out
